# revision 17
# baseline (speedup 1.0000x reference)
"""Trainium2 Bass kernel for nn_DecoderBlock (linear-attention decoder block).

Sharding: token-parallel across 8 cores (each core owns (B*T)/8 = 256 rows of
the flattened [B*T, C] token stream; weights replicated per core). The linear
attention is computed exactly via an intra-chunk causal block plus cross-core
KV prefix states; one small AllGather (~270KB/rank) carries per-core KV states
and Kf sums for both the causal self-attention and the (non-causal)
cross-attention. Activations are kept transposed ([C partitions, tokens free])
so every GEMM lhsT is a plain DRAM weight slice.

Precision: the five attention-side GEMMs (qkv, ca_kv, sa_proj, ca_q, ca_proj)
run in fp8e4 with DoubleRow perf mode (2 K-rows/cycle); the MLP GEMMs (fc,
fcp) stay fp16 for accuracy. Activations quantize with fixed power-of-2
scales (ALPHA_*), weights with a fixed x1024 scale; descales fold into the
PSUM evictions. Small matmuls (LN stat broadcasts, attention denominator
broadcasts, KV states) are batched into full-width PE ops; nothing runs on
gpsimd except the collective trigger (gpsimd semaphores cost ~1.5us each).

Self-contained: only needs numpy + the concourse (Bass) runtime environment.
"""

import math
import numpy as np
from dataclasses import dataclass

P = 128
HD = 64  # head dim (fixed: C // n_head)
LN_EPS = 1e-5

W8S = 1024.0   # fp8 weight scale (w*1024; |w|<0.23 guaranteed for N(0,0.02))
AH = 16.0      # fp8 activation scale for LN outputs (|h| < 6)
AM = 32.0      # fp8 activation scale for memory (|m| < 5.5)
AY = 32.0      # fp8 activation scale for attention outputs (|y| < 5)


@dataclass(frozen=True)
class Cfg:
    B: int = 2
    T: int = 1024
    C: int = 1024
    H: int = 16
    NCORE: int = 8
    gelu: str = "table"
    debug_dump: bool = False

    @property
    def R(self):
        return self.B * self.T // self.NCORE

    @property
    def KC(self):
        return self.C // P

    @property
    def NT(self):
        return math.ceil(self.R / P)

    @property
    def NPAIR(self):
        return self.H // 2

    @property
    def AGW(self):
        return 2 * (HD * self.NPAIR + self.NPAIR)


# ---------------------------------------------------------------------------
# Host-side helpers
# ---------------------------------------------------------------------------

def _rope_tables(T):
    inv = 1.0 / (10000.0 ** (np.arange(0, HD, 2, dtype=np.float64) / HD))
    freqs = np.outer(np.arange(T), inv)
    emb = np.concatenate([freqs, freqs], axis=-1)
    return np.cos(emb).astype(np.float32), np.sin(emb).astype(np.float32)


def _pack_cols(vecs):
    flat = np.concatenate([np.asarray(v, np.float32).ravel() for v in vecs])
    assert flat.size % P == 0
    return np.ascontiguousarray(flat.reshape(-1, P).T)


def _q8w(w):
    import ml_dtypes
    w = np.asarray(w, np.float32) * W8S
    return np.ascontiguousarray(
        np.clip(w, -240.0, 240.0).astype(ml_dtypes.float8_e4m3))


def _host_inputs(cfg: Cfg, inputs):
    B, T, C, NC = cfg.B, cfg.T, cfg.C, cfg.NCORE
    R = cfg.R
    xf = np.ascontiguousarray(np.asarray(inputs["x"], np.float32).reshape(B * T, C))
    mf = np.ascontiguousarray(np.asarray(inputs["memory"], np.float32).reshape(B * T, C))
    cos, sin = _rope_tables(T)

    # ln1/ln2 gamma+beta are pre-scaled by AH so the LN eviction emits fp8
    # h*AH directly.
    params = _pack_cols([
        np.asarray(inputs["ln1_g"], np.float32) * AH,
        np.asarray(inputs["ln1_b"], np.float32) * AH,
        np.asarray(inputs["ln2_g"], np.float32) * AH,
        np.asarray(inputs["ln2_b"], np.float32) * AH,
        inputs["ln3_g"], inputs["ln3_b"],
        inputs["sa_qkv_b"], inputs["sa_proj_b"], inputs["ca_q_b"],
        inputs["ca_kv_b"], inputs["ca_proj_b"],
        inputs["fc_b"], inputs["fcp_b"]])

    maskT = np.ascontiguousarray(np.triu(np.ones((R, R), np.float32)))
    ea = np.zeros((2, P), np.float32)
    ea[0, :HD] = 1.0
    ea[1, HD:] = 1.0

    weights = {}
    for k in ("sa_qkv_w", "ca_kv_w", "sa_proj_w", "ca_q_w", "ca_proj_w"):
        weights[k] = _q8w(inputs[k])
    for k in ("fc_w", "fcp_w"):
        weights[k] = np.ascontiguousarray(np.asarray(inputs[k]).astype(np.float16))

    cpb = NC // B
    in_maps = []
    for c in range(NC):
        r0 = c * R
        pos = np.arange(r0, r0 + R) % T
        cos2 = np.ascontiguousarray(np.vstack([cos[pos].T, cos[pos].T]))
        sin2 = np.ascontiguousarray(np.vstack([sin[pos].T, sin[pos].T]))
        b = c // cpb
        wpre = np.array([1.0 if (r // cpb == b and r < c) else 0.0
                         for r in range(NC)], np.float32)
        wtot = np.array([1.0 if r // cpb == b else 0.0
                         for r in range(NC)], np.float32)
        wsel = np.ascontiguousarray(
            np.tile(np.concatenate([wpre, wtot])[None, :], (P, 1)).astype(np.float32))
        m = dict(weights)
        m.update({
            "x_c": xf[r0:r0 + R].copy(),
            "m_c": mf[r0:r0 + R].copy(),
            "cos2": cos2, "sin2": sin2, "maskT": maskT,
            "wsel": wsel, "params": params, "ea": ea,
        })
        in_maps.append(m)
    return in_maps


# ---------------------------------------------------------------------------
# Bass program
# ---------------------------------------------------------------------------

def build_program(cfg: Cfg):
    import concourse.bass as bass
    import concourse.mybir as mybir
    import concourse.tile as tile
    from concourse import bacc
    from concourse.masks import make_identity
    from contextlib import ExitStack

    dt = mybir.dt
    f32 = dt.float32
    f16 = dt.float16
    f8 = dt.float8e4
    bf16 = dt.bfloat16
    AF = mybir.ActivationFunctionType
    OP = mybir.AluOpType
    AX = mybir.AxisListType
    DR = mybir.MatmulPerfMode.DoubleRow

    B, T, C, H, NC = cfg.B, cfg.T, cfg.C, cfg.H, cfg.NCORE
    R, KC, NT, NPAIR, AGW = cfg.R, cfg.KC, cfg.NT, cfg.NPAIR, cfg.AGW
    KP = KC // 2          # k-tile pairs for fp8 DoubleRow
    RT = [min(P, R - n * P) for n in range(NT)]
    SPW = max(2 * R, P)
    GW = 4  # GEMM m-group width (PSUM banks)

    DSC_QKV = 1.0 / (W8S * AH)
    DSC_CAKV = 1.0 / (W8S * AM)
    DSC_SAP = 1.0 / (W8S * AY)
    DSC_CAQ = 1.0 / (W8S * AH)
    DSC_CAP = 1.0 / (W8S * AY)

    nc = bacc.Bacc("TRN2", target_bir_lowering=False, debug=False,
                   num_devices=cfg.NCORE)

    x_c = nc.dram_tensor("x_c", [R, C], f32, kind="ExternalInput")
    m_c = nc.dram_tensor("m_c", [R, C], f32, kind="ExternalInput")
    cos2_d = nc.dram_tensor("cos2", [P, R], f32, kind="ExternalInput")
    sin2_d = nc.dram_tensor("sin2", [P, R], f32, kind="ExternalInput")
    maskT_d = nc.dram_tensor("maskT", [R, R], f32, kind="ExternalInput")
    wsel_d = nc.dram_tensor("wsel", [P, 2 * NC], f32, kind="ExternalInput")
    ea_d = nc.dram_tensor("ea", [2, P], f32, kind="ExternalInput")
    NPCOL = 19 * KC
    params_d = nc.dram_tensor("params", [P, NPCOL], f32, kind="ExternalInput")
    Wqkv = nc.dram_tensor("sa_qkv_w", [C, 3 * C], f8, kind="ExternalInput")
    Wsap = nc.dram_tensor("sa_proj_w", [C, C], f8, kind="ExternalInput")
    Wcaq = nc.dram_tensor("ca_q_w", [C, C], f8, kind="ExternalInput")
    Wcakv = nc.dram_tensor("ca_kv_w", [C, 2 * C], f8, kind="ExternalInput")
    Wcap = nc.dram_tensor("ca_proj_w", [C, C], f8, kind="ExternalInput")
    Wfc = nc.dram_tensor("fc_w", [C, 4 * C], f16, kind="ExternalInput")
    Wfcp = nc.dram_tensor("fcp_w", [4 * C, C], f16, kind="ExternalInput")
    out_d = nc.dram_tensor("out", [R, C], f32, kind="ExternalOutput")

    off = {}
    cur = 0
    for pname, w in (("ln1_g", KC), ("ln1_b", KC), ("ln2_g", KC), ("ln2_b", KC),
                     ("ln3_g", KC), ("ln3_b", KC), ("qkv_b", 3 * KC),
                     ("sap_b", KC), ("caq_b", KC), ("cakv_b", 2 * KC),
                     ("cap_b", KC), ("fc_b", 4 * KC), ("fcp_b", KC)):
        off[pname] = cur
        cur += w
    assert cur == NPCOL

    with tile.TileContext(nc) as tc, ExitStack() as ctx:
        const = ctx.enter_context(tc.tile_pool(name="const", bufs=1))
        act = ctx.enter_context(tc.tile_pool(name="act", bufs=1))
        wpool = ctx.enter_context(tc.tile_pool(name="wpool", bufs=6))
        tmp = ctx.enter_context(tc.tile_pool(name="tmp", bufs=2))
        gps = ctx.enter_context(tc.tile_pool(name="gps", bufs=GW, space="PSUM"))
        sps = ctx.enter_context(tc.tile_pool(name="sps", bufs=4, space="PSUM"))
        dram = ctx.enter_context(tc.tile_pool(name="dram", bufs=1, space="DRAM"))

        ident = const.tile([P, P], f32, name="ident")
        make_identity(nc, ident)
        identm = const.tile([P, P], f16, name="identm")
        nc.scalar.copy(identm[:], ident[:])
        params = const.tile([P, NPCOL], f32, name="params")
        nc.sync.dma_start(params[:], params_d[:, :])
        wsel = const.tile([P, 2 * NC], f32, name="wsel")
        nc.sync.dma_start(wsel[:], wsel_d[:, :])
        ones = const.tile([P, 1], f32, name="ones")
        nc.vector.memset(ones[:], 1.0)
        ones116 = const.tile([1, P], f16, name="ones116")
        nc.vector.memset(ones116[:], 1.0)
        ones16 = const.tile([P, 1], f16, name="ones16")
        nc.vector.memset(ones16[:], 1.0)
        # Ea: [2,128] expander; row0 -> partitions 0:64, row1 -> 64:128
        Eaf = const.tile([2, P], f32, name="Eaf")
        nc.sync.dma_start(Eaf[:], ea_d[:, :])
        Ea = const.tile([2, P], f16, name="Ea")
        nc.scalar.copy(Ea[:], Eaf[:])
        epsT = const.tile([1, 1], f32, name="epsT")
        nc.vector.memset(epsT[:], LN_EPS)
        maskT = []
        for n in range(NT):
            mt = const.tile([P, R], f32, name=f"maskT{n}")
            nc.sync.dma_start(mt[:RT[n], :], maskT_d[n * P:n * P + RT[n], :])
            maskT.append(mt)
        cosR = const.tile([P, R], f32, name="cosR")
        nc.sync.dma_start(cosR[:], cos2_d[:, :])
        sinR = const.tile([P, R], f32, name="sinR")
        nc.sync.dma_start(sinR[:], sin2_d[:, :])
        cosW = const.tile([P, KC, R], f16, name="cosW")
        sinW = const.tile([P, KC, R], f16, name="sinW")
        for k in range(KC):
            nc.scalar.copy(cosW[:, k, :], cosR[:])
            nc.scalar.copy(sinW[:, k, :], sinR[:])

        def pcol(pname, j):
            return params[:, off[pname] + j:off[pname] + j + 1]

        # ---- load [R, C] natural -> transposed tiles ----
        def load_transposed_f32(src_dram, names, bufs=1):
            tiles = [act.tile([P, R], f32, name=names(k), bufs=bufs)
                     for k in range(KC)]
            for n in range(NT):
                nat = tmp.tile([P, C], f32, name="nat", bufs=2)
                nc.sync.dma_start(nat[:RT[n], :], src_dram[n * P:n * P + RT[n], :])
                for k in range(KC):
                    pt = sps.tile([P, SPW], f32, name="sps")
                    nc.tensor.transpose(pt[:P, :RT[n]],
                                        nat[:RT[n], k * P:(k + 1) * P],
                                        ident[:RT[n], :RT[n]])
                    nc.scalar.copy(tiles[k][:, n * P:n * P + RT[n]],
                                   pt[:P, :RT[n]])
            return tiles

        def load_transposed_q8(src_dram, names, alpha):
            # paired [P, 2, R] fp8 tiles (DoubleRow rhs layout), scaled alpha
            tiles = [act.tile([P, 2, R], f8, name=names(kp)) for kp in range(KP)]
            for n in range(NT):
                nat = tmp.tile([P, C], f32, name="nat", bufs=2)
                nc.sync.dma_start(nat[:RT[n], :], src_dram[n * P:n * P + RT[n], :])
                for k in range(KC):
                    pt = sps.tile([P, SPW], f32, name="sps")
                    nc.tensor.transpose(pt[:P, :RT[n]],
                                        nat[:RT[n], k * P:(k + 1) * P],
                                        ident[:RT[n], :RT[n]])
                    nc.scalar.mul(tiles[k // 2][:, k % 2, n * P:n * P + RT[n]],
                                  pt[:P, :RT[n]], alpha)
            return tiles

        mT = load_transposed_q8(m_c, lambda kp: f"mm{kp}", AM)
        xT = load_transposed_f32(x_c, lambda k: f"res{k}", bufs=2)

        # ---- layernorm on transposed activations ----
        # pairs=True: emit 4 [P,2,R] fp8 tiles (alpha pre-folded into params);
        # else 8 [P,R] tiles of dtype odt.
        def layernorm(xt, gname, bname, pairs, odt):
            ps_mu = sps.tile([P, SPW], f32, name="sps")
            ps_sq = sps.tile([P, SPW], f32, name="sps")
            for k in range(KC):
                xf = tmp.tile([P, R], f16, name="lnxf", bufs=2)
                nc.vector.tensor_copy(xf[:], xt[k][:])
                sq = tmp.tile([P, R], f16, name="lnsq", bufs=2)
                nc.scalar.square(sq[:], xt[k][:])
                nc.tensor.matmul(ps_mu[0:1, :R], lhsT=ones16[:], rhs=xf[:],
                                 start=(k == 0), stop=(k == KC - 1))
                nc.tensor.matmul(ps_sq[0:1, :R], lhsT=ones16[:], rhs=sq[:],
                                 start=(k == 0), stop=(k == KC - 1))
            mu = tmp.tile([1, R], f32, name="ln_mu", bufs=1)
            nc.scalar.mul(mu[:], ps_mu[0:1, :R], 1.0 / C)
            ex2 = tmp.tile([1, R], f32, name="ln_ex2", bufs=1)
            nc.scalar.mul(ex2[:], ps_sq[0:1, :R], 1.0 / C)
            mu2 = tmp.tile([1, R], f32, name="ln_mu2", bufs=1)
            nc.scalar.square(mu2[:], mu[:])
            var = tmp.tile([1, R], f32, name="ln_var", bufs=1)
            nc.vector.tensor_sub(var[:], ex2[:], mu2[:])
            std = tmp.tile([1, R], f32, name="ln_std", bufs=1)
            nc.scalar.activation(std[:], var[:], AF.Sqrt, bias=epsT[:])
            rstd = tmp.tile([1, R], f32, name="ln_rstd", bufs=1)
            nc.vector.reciprocal_approx_fast(rstd[:], std[:])
            mu16 = tmp.tile([1, R], f16, name="ln_mu16", bufs=1)
            nc.scalar.copy(mu16[:], mu[:])
            rstd16 = tmp.tile([1, R], f16, name="ln_rstd16", bufs=1)
            nc.scalar.copy(rstd16[:], rstd[:])
            mub = sps.tile([P, SPW], f32, name="sps")
            nc.tensor.matmul(mub[:, :R], lhsT=ones116[:], rhs=mu16[:],
                             start=True, stop=True)
            rstdb = sps.tile([P, SPW], f32, name="sps")
            nc.tensor.matmul(rstdb[:, :R], lhsT=ones116[:], rhs=rstd16[:],
                             start=True, stop=True)
            if pairs:
                hs = [act.tile([P, 2, R], f8, name=f"h{kp}", bufs=2)
                      for kp in range(KP)]
            else:
                hs = [act.tile([P, R], odt, name=f"h{k}", bufs=2)
                      for k in range(KC)]
            for k in range(KC):
                t1 = tmp.tile([P, R], f32, name="ln_cen", bufs=2)
                nc.vector.tensor_sub(t1[:], xt[k][:], mub[:, :R])
                nc.vector.tensor_mul(t1[:], t1[:], rstdb[:, :R])
                dst = hs[k // 2][:, k % 2, :] if pairs else hs[k][:]
                nc.vector.tensor_scalar(dst, t1[:], pcol(gname, k),
                                        pcol(bname, k), op0=OP.mult, op1=OP.add)
            return hs

        # ---- GEMM (fp16 rhs tiles): out[M=F, N=R] = W^T @ rhs ----
        def gemm16(w_dram, rhs_tiles, F, evict):
            KT = len(rhs_tiles)
            MT = F // P
            for gi, g0 in enumerate(range(0, MT, GW)):
                gl = min(GW, MT - g0)
                pool = gps if gi % 2 == 0 else sps
                pss = [pool.tile([P, SPW], f32, name="gps" if gi % 2 == 0 else "sps")
                       for _ in range(gl)]
                for k in range(KT):
                    wt = wpool.tile([P, GW * P], f16, name="wt")
                    nc.sync.dma_start(
                        wt[:, :gl * P],
                        w_dram[k * P:(k + 1) * P, g0 * P:(g0 + gl) * P])
                    for j in range(gl):
                        nc.tensor.matmul(
                            pss[j][:, :R],
                            lhsT=wt[:, j * P:(j + 1) * P],
                            rhs=rhs_tiles[k][:],
                            start=(k == 0), stop=(k == KT - 1))
                for j in range(gl):
                    evict(g0 + j, pss[j][:, :R])

        # ---- GEMM (fp8 DoubleRow): rhs_pairs = KP tiles [P, 2, R] fp8 ----
        def gemm8(w_dram, rhs_pairs, F, evict, group_order=None):
            MT = F // P
            gidx = list(range(0, MT, GW))
            if group_order is not None:
                gidx = [gidx[i] for i in group_order]
            for gi, g0 in enumerate(gidx):
                gl = min(GW, MT - g0)
                pool = gps if gi % 2 == 0 else sps
                pss = [pool.tile([P, SPW], f32, name="gps" if gi % 2 == 0 else "sps")
                       for _ in range(gl)]
                for kp in range(KP):
                    wt = wpool.tile([P, 2, GW * P], f8, name="wt8")
                    nc.sync.dma_start(
                        wt[:, 0, :gl * P],
                        w_dram[2 * kp * P:(2 * kp + 1) * P, g0 * P:(g0 + gl) * P])
                    nc.sync.dma_start(
                        wt[:, 1, :gl * P],
                        w_dram[(2 * kp + 1) * P:(2 * kp + 2) * P,
                               g0 * P:(g0 + gl) * P])
                    for j in range(gl):
                        nc.tensor.matmul(
                            pss[j][:, :R],
                            lhsT=wt[:, :, j * P:(j + 1) * P],
                            rhs=rhs_pairs[kp][:, :, :],
                            start=(kp == 0), stop=(kp == KP - 1),
                            perf_mode=DR)
                for j in range(gl):
                    evict(g0 + j, pss[j][:, :R])

        # ---- elementwise helpers (head-pair packed [128, R] tiles) ----
        HKC = KC // 2

        def elu1_w(srcw, ko):
            # ELU(x)+1 = exp(min(x,0)) + relu(x), 4 pairs per instruction;
            # overwrites srcw[:, ko:ko+HKC, :] (raw GEMM output is dead after)
            s = srcw[:, ko:ko + HKC, :]
            mn = tmp.tile([P, HKC, R], f32, name="ew_mn", bufs=2)
            nc.vector.tensor_scalar_min(mn[:], s, 0.0)
            ex = tmp.tile([P, HKC, R], f32, name="ew_ex", bufs=2)
            nc.scalar.activation(ex[:], mn[:], AF.Exp)
            mx = tmp.tile([P, HKC, R], f32, name="ew_mx", bufs=2)
            nc.scalar.activation(mx[:], s, AF.Relu)
            nc.vector.tensor_add(s, ex[:], mx[:])

        def rope_w(srcf, ko, dst):
            s = srcf[:, ko:ko + HKC, :]
            rot = tmp.tile([P, HKC, R], f32, name="ew_mn", bufs=2)
            hh = HD // 2
            for h0 in (0, HD):
                nc.scalar.mul(rot[h0:h0 + hh, :, :],
                              srcf[h0 + hh:h0 + HD, ko:ko + HKC, :], -1.0)
                nc.scalar.copy(rot[h0 + hh:h0 + HD, :, :],
                               srcf[h0:h0 + hh, ko:ko + HKC, :])
            a = tmp.tile([P, HKC, R], f32, name="ew_ex", bufs=2)
            nc.vector.tensor_mul(a[:], s, cosW[:, ko:ko + HKC, :])
            nc.vector.tensor_mul(rot[:], rot[:], sinW[:, ko:ko + HKC, :])
            nc.vector.tensor_add(dst, a[:], rot[:])

        def transpose_pair(slc, names, dtype, bufs=1):
            # slc(n) -> [128, RT[n]] AP of the pair's packed [hd, s] block
            outs = []
            for n in range(NT):
                pt = sps.tile([P, SPW], dtype, name="sps")
                nc.tensor.transpose(pt[:RT[n], :P], slc(n), identm[:, :])
                o = tmp.tile([P, P], dtype, name=names(n), bufs=bufs)
                if n % 2 == 0:
                    nc.scalar.copy(o[:RT[n], :], pt[:RT[n], :P])
                else:
                    nc.vector.tensor_copy(o[:RT[n], :], pt[:RT[n], :P])
                outs.append(o)
            return outs

        def kv_state(Kn, Vn, dst_ap):
            # one full [128,128] matmul per chunk: diag 64x64 blocks are the
            # per-head states, off-diag blocks are discarded
            st = sps.tile([P, SPW], f32, name="sps")
            for n in range(NT):
                nc.tensor.matmul(st[:, :P],
                                 lhsT=Kn[n][:RT[n], :], rhs=Vn[n][:RT[n], :],
                                 start=(n == 0), stop=(n == NT - 1))
            nc.scalar.copy(dst_ap[0:HD, :], st[0:HD, 0:HD])
            nc.scalar.copy(dst_ap[HD:P, :], st[HD:P, HD:P])

        dbg_tensors = {}

        def dump(name, tiles_or_ap):
            if not cfg.debug_dump:
                return
            if isinstance(tiles_or_ap, list):
                dd = nc.dram_tensor(f"dbg_{name}",
                                    [len(tiles_or_ap) * P, R], f32,
                                    kind="ExternalOutput")
                for i, t in enumerate(tiles_or_ap):
                    if t.dtype != f32:
                        cpy = tmp.tile([P, R], f32, name="dbgc", bufs=2)
                        nc.vector.tensor_copy(cpy[:], t[:])
                        t = cpy
                    nc.sync.dma_start(dd[i * P:(i + 1) * P, :], t[:])
            else:
                ap = tiles_or_ap
                dd = nc.dram_tensor(f"dbg_{name}", list(ap.shape), f32,
                                    kind="ExternalOutput")
                if ap.dtype != f32:
                    cpy = tmp.tile(list(ap.shape), f32, name="dbgc2", bufs=2)
                    nc.vector.tensor_copy(cpy[:], ap)
                    ap = cpy[:]
                nc.sync.dma_start(dd[:, :], ap)

        # ================= phase 1: cross kv + qkv + states =================

        go = {}

        def evict_store(base, bname, descale, dst_for=None):
            def ev(m, ps):
                dst = dst_for(m) if dst_for is not None else None
                if dst is None:
                    d = act.tile([P, R], f16, name=f"go{base + m}")
                    go[base + m] = d
                    dst = d[:]
                nc.vector.tensor_scalar(dst, ps, descale, pcol(bname, m),
                                        op0=OP.mult, op1=OP.add)
            return ev

        wideK2 = act.tile([P, KC, R], f16, name="wideK2")
        gemm8(Wcakv, mT, 2 * C,
              evict_store(3 * KC, "cakv_b", DSC_CAKV,
                          dst_for=lambda m: wideK2[:, m, :] if m < KC else None))
        h1 = layernorm(xT, "ln1_g", "ln1_b", pairs=True, odt=f8)
        wideQ = act.tile([P, KC, R], f16, name="wideQ")
        wideK = act.tile([P, KC, R], f16, name="wideK")
        gemm8(Wqkv, h1, 3 * C,
              evict_store(0, "qkv_b", DSC_QKV,
                          dst_for=lambda m: wideQ[:, m, :] if m < KC
                          else (wideK[:, m - KC, :] if m < 2 * KC else None)),
              group_order=[2, 3, 4, 5, 0, 1])

        agbuf = act.tile([P, AGW], bf16, name="agbuf")
        o_sst, o_skf = 0, HD * NPAIR
        base2 = HD * NPAIR + NPAIR
        o_cst, o_ckf = base2, base2 + HD * NPAIR

        Kr_w = act.tile([P, KC, R], f16, name="Kr_w")
        K2r_w = act.tile([P, KC, R], f16, name="K2r_w")
        kfsW = tmp.tile([P, KC], f32, name="kfsW", bufs=1)
        kfsW2 = tmp.tile([P, KC], f32, name="kfsW2", bufs=1)
        Vn_l = [None] * NPAIR
        for h in (0, 1):
            ko = h * HKC
            elu1_w(wideK, ko)
            nc.vector.reduce_sum(kfsW[:, ko:ko + HKC],
                                 wideK[:, ko:ko + HKC, :], axis=AX.X)
            rope_w(wideK, ko, Kr_w[:, ko:ko + HKC, :])
            for p in range(ko, ko + HKC):
                Vn_l[p] = transpose_pair(
                    lambda n: go[2 * KC + p][:, n * P:n * P + RT[n]],
                    lambda n: f"Vn{p}_{n}", f16)
                Kn = transpose_pair(
                    lambda n: Kr_w[:, p, n * P:n * P + RT[n]],
                    lambda n: "t_kn", f16, bufs=2)
                kv_state(Kn, Vn_l[p],
                         agbuf[:, o_sst + p * HD:o_sst + (p + 1) * HD])
        nc.scalar.copy(agbuf[:, o_skf:o_skf + KC], kfsW[:])
        AGS = HD * NPAIR + NPAIR
        ag_in1 = dram.tile([P, AGS], bf16, name="ag_in1")
        ag_out1 = dram.tile([NC * P, AGS], bf16, name="ag_out1",
                            addr_space="Shared")
        nc.sync.dma_start(ag_in1[:], agbuf[:, 0:AGS])
        nc.gpsimd.collective_compute(
            "AllGather", OP.bypass,
            replica_groups=[list(range(NC))],
            ins=[ag_in1[:].opt()], outs=[ag_out1[:].opt()])
        for h in (0, 1):
            ko = h * HKC
            elu1_w(wideK2, ko)
            nc.vector.reduce_sum(kfsW2[:, ko:ko + HKC],
                                 wideK2[:, ko:ko + HKC, :], axis=AX.X)
            rope_w(wideK2, ko, K2r_w[:, ko:ko + HKC, :])
            for p in range(ko, ko + HKC):
                V2n = transpose_pair(
                    lambda n: go[4 * KC + p][:, n * P:n * P + RT[n]],
                    lambda n: "t_v2n", f16, bufs=2)
                K2n = transpose_pair(
                    lambda n: K2r_w[:, p, n * P:n * P + RT[n]],
                    lambda n: "t_kn", f16, bufs=2)
                kv_state(K2n, V2n,
                         agbuf[:, o_cst + p * HD:o_cst + (p + 1) * HD])
        nc.scalar.copy(agbuf[:, o_ckf:o_ckf + KC], kfsW2[:])

        # ---------- pre-AG: Q features + intra causal attention ----------
        Qr_w = act.tile([P, KC, R], f16, name="Qr_w")
        for h in (0, 1):
            elu1_w(wideQ, h * HKC)
            rope_w(wideQ, h * HKC, Qr_w[:, h * HKC:(h + 1) * HKC, :])
        Qf_w = wideQ
        yi_l = [None] * NPAIR
        for p in range(NPAIR):
            yp = gps.tile([P, SPW], f32, name="gps")
            ams = {}
            for h0 in (0, HD):
                for n in range(NT):
                    pa = sps.tile([P, SPW], f32, name="sps")
                    nc.tensor.matmul(
                        pa[:RT[n], :R],
                        lhsT=Kr_w[h0:h0 + HD, p, n * P:n * P + RT[n]],
                        rhs=Qr_w[h0:h0 + HD, p, :],
                        start=True, stop=True)
                    am = tmp.tile([P, R], f16, name="attM", bufs=4)
                    nc.vector.tensor_mul(am[:RT[n], :], pa[:RT[n], :R],
                                         maskT[n][:RT[n], :])
                    ams[(h0, n)] = am
            for h0 in (0, HD):
                for n in range(NT):
                    nc.tensor.matmul(
                        yp[h0:h0 + HD, :R],
                        lhsT=Vn_l[p][n][:RT[n], h0:h0 + HD],
                        rhs=ams[(h0, n)][:RT[n], :],
                        start=(n == 0), stop=(n == NT - 1))
            yi = act.tile([P, R], f32, name=f"yi{p}")
            nc.scalar.copy(yi[:], yp[:, :R])
            yi_l[p] = yi

        # ============ AllGather part 2 (CA region) + reduces ============
        ag_in2 = dram.tile([P, AGW - AGS], bf16, name="ag_in2")
        ag_out2 = dram.tile([NC * P, AGW - AGS], bf16, name="ag_out2",
                            addr_space="Shared")
        nc.sync.dma_start(ag_in2[:], agbuf[:, AGS:AGW])
        nc.gpsimd.collective_compute(
            "AllGather", OP.bypass,
            replica_groups=[list(range(NC))],
            ins=[ag_in2[:].opt()], outs=[ag_out2[:].opt()])
        # note: AG-2 is issued here, after the K2 section; AG-1 is already in
        # flight, so the SA reduce below overlaps AG-2's transport.
        OSP = HD * NPAIR
        accP = act.tile([P, AGW], f32, name="accP")
        accT = act.tile([P, AGW], f32, name="accT")
        agr1_l = []
        for r in range(NC):
            agr = tmp.tile([P, AGS], bf16, name=f"agr1_{r}", bufs=1)
            nc.sync.dma_start(agr[:], ag_out1[r * P:(r + 1) * P, :])
            agr1_l.append(agr)
        nc.vector.memset(accP[:, 0:OSP], 0.0)
        nc.vector.memset(accT[:, OSP:AGS], 0.0)
        for r in range(NC):
            nc.vector.scalar_tensor_tensor(accP[:, 0:OSP], agr1_l[r][:, 0:OSP],
                                           wsel[:, r:r + 1],
                                           accP[:, 0:OSP], op0=OP.mult, op1=OP.add)
        for r in range(NC):
            nc.vector.scalar_tensor_tensor(accT[:, OSP:AGS],
                                           agr1_l[r][:, OSP:AGS],
                                           wsel[:, NC + r:NC + r + 1],
                                           accT[:, OSP:AGS],
                                           op0=OP.mult, op1=OP.add)
        agr2_l = []
        for r in range(NC):
            agr = tmp.tile([P, AGW - AGS], bf16, name=f"agr2_{r}", bufs=1)
            nc.sync.dma_start(agr[:], ag_out2[r * P:(r + 1) * P, :])
            agr2_l.append(agr)
        nc.vector.memset(accT[:, AGS:AGW], 0.0)
        for r in range(NC):
            nc.vector.scalar_tensor_tensor(accT[:, AGS:AGW], agr2_l[r][:],
                                           wsel[:, NC + r:NC + r + 1],
                                           accT[:, AGS:AGW],
                                           op0=OP.mult, op1=OP.add)

        accPm = act.tile([P, AGW], f16, name="accPm")
        nc.scalar.copy(accPm[:, 0:OSP], accP[:, 0:OSP])
        accTm = act.tile([P, AGW], f16, name="accTm")
        nc.scalar.copy(accTm[:, OSP:AGS], accT[:, OSP:AGS])
        nc.scalar.copy(accTm[:, AGS:AGW], accT[:, AGS:AGW])

        # kf2: zero-padded per-head-half Kf-sum columns, [128, 2] per pair
        # (SA pairs at cols 2p, CA pairs at cols 2*NPAIR + 2p)
        kf2 = act.tile([P, 4 * NPAIR], f16, name="kf2")
        nc.vector.memset(kf2[:], 0.0)
        for p in range(NPAIR):
            c = o_skf + p
            nc.scalar.copy(kf2[0:HD, 2 * p:2 * p + 1], accTm[0:HD, c:c + 1])
            nc.scalar.copy(kf2[HD:P, 2 * p + 1:2 * p + 2], accTm[HD:P, c:c + 1])
        for p in range(NPAIR):
            c = o_ckf + p
            b = 2 * NPAIR
            nc.scalar.copy(kf2[0:HD, b + 2 * p:b + 2 * p + 1],
                           accTm[0:HD, c:c + 1])
            nc.scalar.copy(kf2[HD:P, b + 2 * p + 1:b + 2 * p + 2],
                           accTm[HD:P, c:c + 1])

        # ================= self attention =================
        def divide_and_pack(yp, Qf, kfbase, dst_ap, add=None):
            # den rows [2, R] = per-half Qf . kf_sum; reciprocal (x AY) then
            # broadcast to [128, R] via the Ea expander matmul
            dps = sps.tile([P, SPW], f32, name="sps")
            nc.tensor.matmul(dps[0:2, :R], lhsT=kf2[:, kfbase:kfbase + 2],
                             rhs=Qf[:], start=True, stop=True)
            rsf = tmp.tile([2, R], f32, name="d_rsf", bufs=2)
            nc.vector.reciprocal_approx_fast(rsf[:], dps[0:2, :R])
            rs16 = tmp.tile([2, R], f16, name="d_rs16", bufs=2)
            nc.scalar.mul(rs16[:], rsf[:], AY)
            denb = sps.tile([P, SPW], f32, name="sps")
            nc.tensor.matmul(denb[:, :R], lhsT=Ea[:], rhs=rs16[:],
                             start=True, stop=True)
            if add is not None:
                ys = tmp.tile([P, R], f32, name="ysum", bufs=2)
                nc.vector.tensor_add(ys[:], yp[:, :R], add[:])
                nc.vector.tensor_mul(dst_ap, ys[:], denb[:, :R])
            else:
                ys = tmp.tile([P, R], f32, name="ysum", bufs=2)
                nc.scalar.copy(ys[:], yp[:, :R])
                nc.vector.tensor_mul(dst_ap, ys[:], denb[:, :R])

        ySA = [act.tile([P, 2, R], f8, name=f"ySA{i}") for i in range(NPAIR // 2)]
        for p in range(NPAIR):
            yp = gps.tile([P, SPW], f32, name="gps")
            for h0 in (0, HD):
                nc.tensor.matmul(
                    yp[h0:h0 + HD, :R],
                    lhsT=accPm[h0:h0 + HD, o_sst + p * HD:o_sst + (p + 1) * HD],
                    rhs=Qr_w[h0:h0 + HD, p, :],
                    start=True, stop=True)
            divide_and_pack(yp, Qf_w[:, p, :], 2 * p,
                            ySA[p // 2][:, p % 2, :], add=yi_l[p])

        x1T = [None] * KC

        def evict_res8(dst, bname, descale, res, rname):
            def ev(m, ps):
                d = act.tile([P, R], f32, name=rname(m), bufs=2)
                t = tmp.tile([P, R], f32, name="ev_t", bufs=2)
                nc.vector.tensor_scalar(t[:], ps, descale, pcol(bname, m),
                                        op0=OP.mult, op1=OP.add)
                nc.vector.tensor_add(d[:], t[:], res[m][:])
                dst[m] = d
            return ev

        gemm8(Wsap, ySA, C, evict_res8(x1T, "sap_b", DSC_SAP, xT,
                                       lambda k: f"res{k}"))
        dump("x1T", x1T)

        # ================= cross attention =================
        h2 = layernorm(x1T, "ln2_g", "ln2_b", pairs=True, odt=f8)
        wideQ2 = act.tile([P, KC, R], f16, name="wideQ2")
        gemm8(Wcaq, h2, C,
              evict_store(4 * KC, "caq_b", DSC_CAQ,
                          dst_for=lambda m: wideQ2[:, m, :]))
        Q2r_w = act.tile([P, KC, R], f16, name="Q2r_w")
        for h in (0, 1):
            elu1_w(wideQ2, h * HKC)
            rope_w(wideQ2, h * HKC, Q2r_w[:, h * HKC:(h + 1) * HKC, :])
        Q2f_w = wideQ2
        yCA = [act.tile([P, 2, R], f8, name=f"yCA{i}") for i in range(NPAIR // 2)]
        for p in range(NPAIR):
            yp = gps.tile([P, SPW], f32, name="gps")
            for h0 in (0, HD):
                nc.tensor.matmul(
                    yp[h0:h0 + HD, :R],
                    lhsT=accTm[h0:h0 + HD, o_cst + p * HD:o_cst + (p + 1) * HD],
                    rhs=Q2r_w[h0:h0 + HD, p, :],
                    start=True, stop=True)
            divide_and_pack(yp, Q2f_w[:, p, :], 2 * NPAIR + 2 * p,
                            yCA[p // 2][:, p % 2, :])

        x2T = [None] * KC
        gemm8(Wcap, yCA, C, evict_res8(x2T, "cap_b", DSC_CAP, x1T,
                                       lambda k: f"res{k}"))
        dump("x2T", x2T)

        # ================= MLP (fp16) =================
        h3 = layernorm(x2T, "ln3_g", "ln3_b", pairs=False, odt=f16)
        gT = [None] * (4 * KC)

        def evict_gelu(m, ps):
            d = act.tile([P, R], f16, name=f"go{m}")
            nc.scalar.activation(d[:], ps, AF.Gelu_apprx_tanh,
                                 bias=pcol("fc_b", m))
            gT[m] = d
        gemm16(Wfc, h3, 4 * C, evict_gelu)

        xoT = [None] * KC

        def evict_res16(dst, bname, res, rname):
            def ev(m, ps):
                d = act.tile([P, R], f32, name=rname(m), bufs=2)
                nc.vector.scalar_tensor_tensor(d[:], ps, pcol(bname, m),
                                               res[m][:], op0=OP.add, op1=OP.add)
                dst[m] = d
            return ev

        gemm16(Wfcp, gT, C, evict_res16(xoT, "fcp_b", x2T, lambda k: f"res{k}"))

        # ================= transpose back + store =================
        for n in range(NT):
            onat = tmp.tile([P, C], f32, name="nat", bufs=2)
            for k in range(KC):
                pt = sps.tile([P, SPW], f32, name="sps")
                nc.tensor.transpose(pt[:RT[n], :P],
                                    xoT[k][:, n * P:n * P + RT[n]],
                                    ident[:, :])
                nc.scalar.copy(onat[:RT[n], k * P:(k + 1) * P],
                               pt[:RT[n], :P])
            nc.sync.dma_start(out_d[n * P:n * P + RT[n], :], onat[:RT[n], :])

    nc.compile()
    return nc


# ---------------------------------------------------------------------------
# Entry point
# ---------------------------------------------------------------------------

_CACHE = {}


def _get_program(cfg: Cfg):
    if cfg not in _CACHE:
        _CACHE[cfg] = build_program(cfg)
    return _CACHE[cfg]


def run(inputs, cfg: Cfg = Cfg(), trace: bool = False):
    from concourse.bass_utils import run_bass_kernel_spmd
    nc = _get_program(cfg)
    in_maps = _host_inputs(cfg, inputs)
    res = run_bass_kernel_spmd(nc, in_maps, core_ids=list(range(cfg.NCORE)),
                               trace=trace)
    outs = [res.results[c]["out"] for c in range(cfg.NCORE)]
    full = np.concatenate(outs, axis=0).reshape(cfg.B, cfg.T, cfg.C)
    return np.asarray(full, np.float32), res


def kernel(**inputs):
    out, _ = run(inputs)
    return out


# revision 18
# speedup vs baseline: 1.0623x; 1.0623x over previous
"""Trainium2 Bass kernel for nn_DecoderBlock (linear-attention decoder block).

Sharding: token-parallel across 8 cores (each core owns (B*T)/8 = 256 rows of
the flattened [B*T, C] token stream; weights replicated per core). The linear
attention is computed exactly via an intra-chunk causal block plus cross-core
KV prefix states; one small AllGather (~270KB/rank) carries per-core KV states
and Kf sums for both the causal self-attention and the (non-causal)
cross-attention. Activations are kept transposed ([C partitions, tokens free])
so every GEMM lhsT is a plain DRAM weight slice.

Precision: the five attention-side GEMMs (qkv, ca_kv, sa_proj, ca_q, ca_proj)
run in fp8e4 with DoubleRow perf mode (2 K-rows/cycle); the MLP GEMMs (fc,
fcp) stay fp16 for accuracy. Activations quantize with fixed power-of-2
scales (ALPHA_*), weights with a fixed x1024 scale; descales fold into the
PSUM evictions. Small matmuls (LN stat broadcasts, attention denominator
broadcasts, KV states) are batched into full-width PE ops; nothing runs on
gpsimd except the collective trigger (gpsimd semaphores cost ~1.5us each).

Self-contained: only needs numpy + the concourse (Bass) runtime environment.
"""

import math
import numpy as np
from dataclasses import dataclass

P = 128
HD = 64  # head dim (fixed: C // n_head)
LN_EPS = 1e-5

W8S = 1024.0   # fp8 weight scale (w*1024; |w|<0.23 guaranteed for N(0,0.02))
AH = 16.0      # fp8 activation scale for LN outputs (|h| < 6)
AM = 32.0      # fp8 activation scale for memory (|m| < 5.5)
AY = 32.0      # fp8 activation scale for attention outputs (|y| < 5)


@dataclass(frozen=True)
class Cfg:
    B: int = 2
    T: int = 1024
    C: int = 1024
    H: int = 16
    NCORE: int = 8
    gelu: str = "table"
    debug_dump: bool = False

    @property
    def R(self):
        return self.B * self.T // self.NCORE

    @property
    def KC(self):
        return self.C // P

    @property
    def NT(self):
        return math.ceil(self.R / P)

    @property
    def NPAIR(self):
        return self.H // 2

    @property
    def AGW(self):
        return 2 * (HD * self.NPAIR + self.NPAIR)


# ---------------------------------------------------------------------------
# Host-side helpers
# ---------------------------------------------------------------------------

def _rope_tables(T):
    inv = 1.0 / (10000.0 ** (np.arange(0, HD, 2, dtype=np.float64) / HD))
    freqs = np.outer(np.arange(T), inv)
    emb = np.concatenate([freqs, freqs], axis=-1)
    return np.cos(emb).astype(np.float32), np.sin(emb).astype(np.float32)


def _pack_cols(vecs):
    flat = np.concatenate([np.asarray(v, np.float32).ravel() for v in vecs])
    assert flat.size % P == 0
    return np.ascontiguousarray(flat.reshape(-1, P).T)


def _q8w(w):
    import ml_dtypes
    w = np.asarray(w, np.float32) * W8S
    return np.ascontiguousarray(
        np.clip(w, -240.0, 240.0).astype(ml_dtypes.float8_e4m3))


def _host_inputs(cfg: Cfg, inputs):
    B, T, C, NC = cfg.B, cfg.T, cfg.C, cfg.NCORE
    R = cfg.R
    xf = np.ascontiguousarray(np.asarray(inputs["x"], np.float32).reshape(B * T, C))
    mf = np.ascontiguousarray(np.asarray(inputs["memory"], np.float32).reshape(B * T, C))
    cos, sin = _rope_tables(T)

    # ln1/ln2 gamma+beta are pre-scaled by AH so the LN eviction emits fp8
    # h*AH directly.
    params = _pack_cols([
        np.asarray(inputs["ln1_g"], np.float32) * AH,
        np.asarray(inputs["ln1_b"], np.float32) * AH,
        np.asarray(inputs["ln2_g"], np.float32) * AH,
        np.asarray(inputs["ln2_b"], np.float32) * AH,
        inputs["ln3_g"], inputs["ln3_b"],
        inputs["sa_qkv_b"], inputs["sa_proj_b"], inputs["ca_q_b"],
        inputs["ca_kv_b"], inputs["ca_proj_b"],
        inputs["fc_b"], inputs["fcp_b"]])

    maskT = np.ascontiguousarray(np.triu(np.ones((R, R), np.float32)))
    ea = np.zeros((2, P), np.float32)
    ea[0, :HD] = 1.0
    ea[1, HD:] = 1.0

    weights = {}
    for k in ("sa_qkv_w", "ca_kv_w", "sa_proj_w", "ca_q_w", "ca_proj_w"):
        weights[k] = _q8w(inputs[k])
    for k in ("fc_w", "fcp_w"):
        weights[k] = np.ascontiguousarray(np.asarray(inputs[k]).astype(np.float16))

    cpb = NC // B
    in_maps = []
    for c in range(NC):
        r0 = c * R
        pos = np.arange(r0, r0 + R) % T
        cos2 = np.ascontiguousarray(np.vstack([cos[pos].T, cos[pos].T]))
        sin2 = np.ascontiguousarray(np.vstack([sin[pos].T, sin[pos].T]))
        b = c // cpb
        wpre = np.array([1.0 if (r // cpb == b and r < c) else 0.0
                         for r in range(NC)], np.float32)
        wtot = np.array([1.0 if r // cpb == b else 0.0
                         for r in range(NC)], np.float32)
        wsel = np.ascontiguousarray(
            np.tile(np.concatenate([wpre, wtot])[None, :], (P, 1)).astype(np.float32))
        m = dict(weights)
        m.update({
            "x_c": xf[r0:r0 + R].copy(),
            "m_c": mf[r0:r0 + R].copy(),
            "cos2": cos2, "sin2": sin2, "maskT": maskT,
            "wsel": wsel, "params": params, "ea": ea,
        })
        in_maps.append(m)
    return in_maps


# ---------------------------------------------------------------------------
# Bass program
# ---------------------------------------------------------------------------

def build_program(cfg: Cfg):
    import concourse.bass as bass
    import concourse.mybir as mybir
    import concourse.tile as tile
    from concourse import bacc
    from concourse.masks import make_identity
    from contextlib import ExitStack

    dt = mybir.dt
    f32 = dt.float32
    f16 = dt.float16
    f8 = dt.float8e4
    bf16 = dt.bfloat16
    AF = mybir.ActivationFunctionType
    OP = mybir.AluOpType
    AX = mybir.AxisListType
    DR = mybir.MatmulPerfMode.DoubleRow

    B, T, C, H, NC = cfg.B, cfg.T, cfg.C, cfg.H, cfg.NCORE
    R, KC, NT, NPAIR, AGW = cfg.R, cfg.KC, cfg.NT, cfg.NPAIR, cfg.AGW
    KP = KC // 2          # k-tile pairs for fp8 DoubleRow
    RT = [min(P, R - n * P) for n in range(NT)]
    SPW = max(2 * R, P)
    GW = 4  # GEMM m-group width (PSUM banks)

    DSC_QKV = 1.0 / (W8S * AH)
    DSC_CAKV = 1.0 / (W8S * AM)
    DSC_SAP = 1.0 / (W8S * AY)
    DSC_CAQ = 1.0 / (W8S * AH)
    DSC_CAP = 1.0 / (W8S * AY)

    nc = bacc.Bacc("TRN2", target_bir_lowering=False, debug=False,
                   num_devices=cfg.NCORE)

    x_c = nc.dram_tensor("x_c", [R, C], f32, kind="ExternalInput")
    m_c = nc.dram_tensor("m_c", [R, C], f32, kind="ExternalInput")
    cos2_d = nc.dram_tensor("cos2", [P, R], f32, kind="ExternalInput")
    sin2_d = nc.dram_tensor("sin2", [P, R], f32, kind="ExternalInput")
    maskT_d = nc.dram_tensor("maskT", [R, R], f32, kind="ExternalInput")
    wsel_d = nc.dram_tensor("wsel", [P, 2 * NC], f32, kind="ExternalInput")
    ea_d = nc.dram_tensor("ea", [2, P], f32, kind="ExternalInput")
    NPCOL = 19 * KC
    params_d = nc.dram_tensor("params", [P, NPCOL], f32, kind="ExternalInput")
    Wqkv = nc.dram_tensor("sa_qkv_w", [C, 3 * C], f8, kind="ExternalInput")
    Wsap = nc.dram_tensor("sa_proj_w", [C, C], f8, kind="ExternalInput")
    Wcaq = nc.dram_tensor("ca_q_w", [C, C], f8, kind="ExternalInput")
    Wcakv = nc.dram_tensor("ca_kv_w", [C, 2 * C], f8, kind="ExternalInput")
    Wcap = nc.dram_tensor("ca_proj_w", [C, C], f8, kind="ExternalInput")
    Wfc = nc.dram_tensor("fc_w", [C, 4 * C], f16, kind="ExternalInput")
    Wfcp = nc.dram_tensor("fcp_w", [4 * C, C], f16, kind="ExternalInput")
    out_d = nc.dram_tensor("out", [R, C], f32, kind="ExternalOutput")

    off = {}
    cur = 0
    for pname, w in (("ln1_g", KC), ("ln1_b", KC), ("ln2_g", KC), ("ln2_b", KC),
                     ("ln3_g", KC), ("ln3_b", KC), ("qkv_b", 3 * KC),
                     ("sap_b", KC), ("caq_b", KC), ("cakv_b", 2 * KC),
                     ("cap_b", KC), ("fc_b", 4 * KC), ("fcp_b", KC)):
        off[pname] = cur
        cur += w
    assert cur == NPCOL

    with tile.TileContext(nc) as tc, ExitStack() as ctx:
        const = ctx.enter_context(tc.tile_pool(name="const", bufs=1))
        act = ctx.enter_context(tc.tile_pool(name="act", bufs=1))
        wpool = ctx.enter_context(tc.tile_pool(name="wpool", bufs=6))
        tmp = ctx.enter_context(tc.tile_pool(name="tmp", bufs=2))
        gps = ctx.enter_context(tc.tile_pool(name="gps", bufs=GW, space="PSUM"))
        sps = ctx.enter_context(tc.tile_pool(name="sps", bufs=4, space="PSUM"))
        dram = ctx.enter_context(tc.tile_pool(name="dram", bufs=1, space="DRAM"))

        ident = const.tile([P, P], f32, name="ident")
        make_identity(nc, ident)
        identm = const.tile([P, P], f16, name="identm")
        nc.scalar.copy(identm[:], ident[:])
        params = const.tile([P, NPCOL], f32, name="params")
        nc.sync.dma_start(params[:], params_d[:, :])
        wsel = const.tile([P, 2 * NC], f32, name="wsel")
        nc.sync.dma_start(wsel[:], wsel_d[:, :])
        ones = const.tile([P, 1], f32, name="ones")
        nc.vector.memset(ones[:], 1.0)
        ones116 = const.tile([1, P], f16, name="ones116")
        nc.vector.memset(ones116[:], 1.0)
        ones16 = const.tile([P, 1], f16, name="ones16")
        nc.vector.memset(ones16[:], 1.0)
        # Ea: [2,128] expander; row0 -> partitions 0:64, row1 -> 64:128
        Eaf = const.tile([2, P], f32, name="Eaf")
        nc.sync.dma_start(Eaf[:], ea_d[:, :])
        Ea = const.tile([2, P], f16, name="Ea")
        nc.scalar.copy(Ea[:], Eaf[:])
        epsT = const.tile([1, 1], f32, name="epsT")
        nc.vector.memset(epsT[:], LN_EPS)
        maskT = []
        for n in range(NT):
            mt = const.tile([P, R], f32, name=f"maskT{n}")
            nc.sync.dma_start(mt[:RT[n], :], maskT_d[n * P:n * P + RT[n], :])
            maskT.append(mt)
        cosR = const.tile([P, R], f32, name="cosR")
        nc.sync.dma_start(cosR[:], cos2_d[:, :])
        sinR = const.tile([P, R], f32, name="sinR")
        nc.sync.dma_start(sinR[:], sin2_d[:, :])
        cosW = const.tile([P, KC, R], f16, name="cosW")
        sinW = const.tile([P, KC, R], f16, name="sinW")
        for k in range(KC):
            nc.scalar.copy(cosW[:, k, :], cosR[:])
            nc.scalar.copy(sinW[:, k, :], sinR[:])

        def pcol(pname, j):
            return params[:, off[pname] + j:off[pname] + j + 1]

        # ---- load [R, C] natural -> transposed tiles ----
        def load_transposed_f32(src_dram, names, bufs=1):
            tiles = [act.tile([P, R], f32, name=names(k), bufs=bufs)
                     for k in range(KC)]
            for n in range(NT):
                nat = tmp.tile([P, C], f32, name="nat", bufs=2)
                nc.sync.dma_start(nat[:RT[n], :], src_dram[n * P:n * P + RT[n], :])
                for k in range(KC):
                    pt = sps.tile([P, SPW], f32, name="sps")
                    nc.tensor.transpose(pt[:P, :RT[n]],
                                        nat[:RT[n], k * P:(k + 1) * P],
                                        ident[:RT[n], :RT[n]])
                    nc.scalar.copy(tiles[k][:, n * P:n * P + RT[n]],
                                   pt[:P, :RT[n]])
            return tiles

        def load_transposed_q8(src_dram, names, alpha):
            # paired [P, 2, R] fp8 tiles (DoubleRow rhs layout), scaled alpha
            tiles = [act.tile([P, 2, R], f8, name=names(kp)) for kp in range(KP)]
            for n in range(NT):
                nat = tmp.tile([P, C], f32, name="nat", bufs=2)
                nc.sync.dma_start(nat[:RT[n], :], src_dram[n * P:n * P + RT[n], :])
                for k in range(KC):
                    pt = sps.tile([P, SPW], f32, name="sps")
                    nc.tensor.transpose(pt[:P, :RT[n]],
                                        nat[:RT[n], k * P:(k + 1) * P],
                                        ident[:RT[n], :RT[n]])
                    nc.scalar.mul(tiles[k // 2][:, k % 2, n * P:n * P + RT[n]],
                                  pt[:P, :RT[n]], alpha)
            return tiles

        mT = load_transposed_q8(m_c, lambda kp: f"mm{kp}", AM)
        xT = load_transposed_f32(x_c, lambda k: f"res{k}", bufs=2)

        # ---- layernorm on transposed activations ----
        # pairs=True: emit 4 [P,2,R] fp8 tiles (alpha pre-folded into params);
        # else 8 [P,R] tiles of dtype odt.
        def layernorm(xt, gname, bname, pairs, odt):
            ps_mu = sps.tile([P, SPW], f32, name="sps")
            ps_sq = sps.tile([P, SPW], f32, name="sps")
            for k in range(KC):
                xf = tmp.tile([P, R], f16, name="lnxf", bufs=2)
                nc.vector.tensor_copy(xf[:], xt[k][:])
                sq = tmp.tile([P, R], f16, name="lnsq", bufs=2)
                nc.scalar.square(sq[:], xt[k][:])
                nc.tensor.matmul(ps_mu[0:1, :R], lhsT=ones16[:], rhs=xf[:],
                                 start=(k == 0), stop=(k == KC - 1))
                nc.tensor.matmul(ps_sq[0:1, :R], lhsT=ones16[:], rhs=sq[:],
                                 start=(k == 0), stop=(k == KC - 1))
            mu = tmp.tile([1, R], f32, name="ln_mu", bufs=1)
            nc.scalar.mul(mu[:], ps_mu[0:1, :R], 1.0 / C)
            ex2 = tmp.tile([1, R], f32, name="ln_ex2", bufs=1)
            nc.scalar.mul(ex2[:], ps_sq[0:1, :R], 1.0 / C)
            mu2 = tmp.tile([1, R], f32, name="ln_mu2", bufs=1)
            nc.scalar.square(mu2[:], mu[:])
            var = tmp.tile([1, R], f32, name="ln_var", bufs=1)
            nc.vector.tensor_sub(var[:], ex2[:], mu2[:])
            std = tmp.tile([1, R], f32, name="ln_std", bufs=1)
            nc.scalar.activation(std[:], var[:], AF.Sqrt, bias=epsT[:])
            rstd = tmp.tile([1, R], f32, name="ln_rstd", bufs=1)
            nc.vector.reciprocal_approx_fast(rstd[:], std[:])
            mu16 = tmp.tile([1, R], f16, name="ln_mu16", bufs=1)
            nc.scalar.copy(mu16[:], mu[:])
            rstd16 = tmp.tile([1, R], f16, name="ln_rstd16", bufs=1)
            nc.scalar.copy(rstd16[:], rstd[:])
            mub = sps.tile([P, SPW], f32, name="sps")
            nc.tensor.matmul(mub[:, :R], lhsT=ones116[:], rhs=mu16[:],
                             start=True, stop=True)
            rstdb = sps.tile([P, SPW], f32, name="sps")
            nc.tensor.matmul(rstdb[:, :R], lhsT=ones116[:], rhs=rstd16[:],
                             start=True, stop=True)
            if pairs:
                hs = [act.tile([P, 2, R], f8, name=f"h{kp}", bufs=2)
                      for kp in range(KP)]
            else:
                hs = [act.tile([P, R], odt, name=f"h{k}", bufs=2)
                      for k in range(KC)]
            for k in range(KC):
                t1 = tmp.tile([P, R], f32, name="ln_cen", bufs=2)
                nc.vector.tensor_sub(t1[:], xt[k][:], mub[:, :R])
                nc.vector.tensor_mul(t1[:], t1[:], rstdb[:, :R])
                dst = hs[k // 2][:, k % 2, :] if pairs else hs[k][:]
                nc.vector.tensor_scalar(dst, t1[:], pcol(gname, k),
                                        pcol(bname, k), op0=OP.mult, op1=OP.add)
            return hs

        # ---- GEMM (fp16 rhs tiles): out[M=F, N=R] = W^T @ rhs ----
        def gemm16(w_dram, rhs_tiles, F, evict):
            KT = len(rhs_tiles)
            MT = F // P
            for gi, g0 in enumerate(range(0, MT, GW)):
                gl = min(GW, MT - g0)
                pool = gps if gi % 2 == 0 else sps
                pss = [pool.tile([P, SPW], f32, name="gps" if gi % 2 == 0 else "sps")
                       for _ in range(gl)]
                for k in range(KT):
                    wt = wpool.tile([P, GW * P], f16, name="wt")
                    nc.sync.dma_start(
                        wt[:, :gl * P],
                        w_dram[k * P:(k + 1) * P, g0 * P:(g0 + gl) * P])
                    for j in range(gl):
                        nc.tensor.matmul(
                            pss[j][:, :R],
                            lhsT=wt[:, j * P:(j + 1) * P],
                            rhs=rhs_tiles[k][:],
                            start=(k == 0), stop=(k == KT - 1))
                for j in range(gl):
                    evict(g0 + j, pss[j][:, :R])

        # ---- GEMM (fp8 DoubleRow): rhs_pairs = KP tiles [P, 2, R] fp8 ----
        def gemm8(w_dram, rhs_pairs, F, evict, group_order=None):
            MT = F // P
            gidx = list(range(0, MT, GW))
            if group_order is not None:
                gidx = [gidx[i] for i in group_order]
            for gi, g0 in enumerate(gidx):
                gl = min(GW, MT - g0)
                pool = gps if gi % 2 == 0 else sps
                pss = [pool.tile([P, SPW], f32, name="gps" if gi % 2 == 0 else "sps")
                       for _ in range(gl)]
                for kp in range(KP):
                    wt = wpool.tile([P, 2, GW * P], f8, name="wt8")
                    nc.sync.dma_start(
                        wt[:, 0, :gl * P],
                        w_dram[2 * kp * P:(2 * kp + 1) * P, g0 * P:(g0 + gl) * P])
                    nc.sync.dma_start(
                        wt[:, 1, :gl * P],
                        w_dram[(2 * kp + 1) * P:(2 * kp + 2) * P,
                               g0 * P:(g0 + gl) * P])
                    for j in range(gl):
                        nc.tensor.matmul(
                            pss[j][:, :R],
                            lhsT=wt[:, :, j * P:(j + 1) * P],
                            rhs=rhs_pairs[kp][:, :, :],
                            start=(kp == 0), stop=(kp == KP - 1),
                            perf_mode=DR)
                for j in range(gl):
                    evict(g0 + j, pss[j][:, :R])

        # ---- elementwise helpers (head-pair packed [128, R] tiles) ----
        HKC = KC // 2

        def elu1_w(srcw, ko):
            # ELU(x)+1 = exp(min(x,0)) + relu(x), 4 pairs per instruction;
            # overwrites srcw[:, ko:ko+HKC, :] (raw GEMM output is dead after)
            s = srcw[:, ko:ko + HKC, :]
            mn = tmp.tile([P, HKC, R], f32, name="ew_mn", bufs=2)
            nc.vector.tensor_scalar_min(mn[:], s, 0.0)
            ex = tmp.tile([P, HKC, R], f32, name="ew_ex", bufs=2)
            nc.scalar.activation(ex[:], mn[:], AF.Exp)
            mx = tmp.tile([P, HKC, R], f32, name="ew_mx", bufs=2)
            nc.scalar.activation(mx[:], s, AF.Relu)
            nc.vector.tensor_add(s, ex[:], mx[:])

        def rope_w(srcf, ko, dst):
            s = srcf[:, ko:ko + HKC, :]
            rot = tmp.tile([P, HKC, R], f32, name="ew_mn", bufs=2)
            hh = HD // 2
            for h0 in (0, HD):
                nc.scalar.mul(rot[h0:h0 + hh, :, :],
                              srcf[h0 + hh:h0 + HD, ko:ko + HKC, :], -1.0)
                nc.scalar.copy(rot[h0 + hh:h0 + HD, :, :],
                               srcf[h0:h0 + hh, ko:ko + HKC, :])
            a = tmp.tile([P, HKC, R], f32, name="ew_ex", bufs=2)
            nc.vector.tensor_mul(a[:], s, cosW[:, ko:ko + HKC, :])
            nc.vector.tensor_mul(rot[:], rot[:], sinW[:, ko:ko + HKC, :])
            nc.vector.tensor_add(dst, a[:], rot[:])

        def transpose_pair(slc, names, dtype, bufs=1):
            # slc(n) -> [128, RT[n]] AP of the pair's packed [hd, s] block
            outs = []
            for n in range(NT):
                pt = sps.tile([P, SPW], dtype, name="sps")
                nc.tensor.transpose(pt[:RT[n], :P], slc(n), identm[:, :])
                o = tmp.tile([P, P], dtype, name=names(n), bufs=bufs)
                if n % 2 == 0:
                    nc.scalar.copy(o[:RT[n], :], pt[:RT[n], :P])
                else:
                    nc.vector.tensor_copy(o[:RT[n], :], pt[:RT[n], :P])
                outs.append(o)
            return outs

        def kv_state(Kn, Vn, dst_ap):
            # one full [128,128] matmul per chunk: diag 64x64 blocks are the
            # per-head states, off-diag blocks are discarded
            st = sps.tile([P, SPW], f32, name="sps")
            for n in range(NT):
                nc.tensor.matmul(st[:, :P],
                                 lhsT=Kn[n][:RT[n], :], rhs=Vn[n][:RT[n], :],
                                 start=(n == 0), stop=(n == NT - 1))
            nc.scalar.copy(dst_ap[0:HD, :], st[0:HD, 0:HD])
            nc.scalar.copy(dst_ap[HD:P, :], st[HD:P, HD:P])

        dbg_tensors = {}

        def dump(name, tiles_or_ap):
            if not cfg.debug_dump:
                return
            if isinstance(tiles_or_ap, list):
                dd = nc.dram_tensor(f"dbg_{name}",
                                    [len(tiles_or_ap) * P, R], f32,
                                    kind="ExternalOutput")
                for i, t in enumerate(tiles_or_ap):
                    if t.dtype != f32:
                        cpy = tmp.tile([P, R], f32, name="dbgc", bufs=2)
                        nc.vector.tensor_copy(cpy[:], t[:])
                        t = cpy
                    nc.sync.dma_start(dd[i * P:(i + 1) * P, :], t[:])
            else:
                ap = tiles_or_ap
                dd = nc.dram_tensor(f"dbg_{name}", list(ap.shape), f32,
                                    kind="ExternalOutput")
                if ap.dtype != f32:
                    cpy = tmp.tile(list(ap.shape), f32, name="dbgc2", bufs=2)
                    nc.vector.tensor_copy(cpy[:], ap)
                    ap = cpy[:]
                nc.sync.dma_start(dd[:, :], ap)

        # ================= phase 1: cross kv + qkv + states =================

        go = {}

        def evict_store(base, bname, descale, dst_for=None):
            def ev(m, ps):
                dst = dst_for(m) if dst_for is not None else None
                if dst is None:
                    d = act.tile([P, R], f16, name=f"go{base + m}")
                    go[base + m] = d
                    dst = d[:]
                nc.vector.tensor_scalar(dst, ps, descale, pcol(bname, m),
                                        op0=OP.mult, op1=OP.add)
            return ev

        wideK2 = act.tile([P, KC, R], f16, name="wideK2")
        gemm8(Wcakv, mT, 2 * C,
              evict_store(3 * KC, "cakv_b", DSC_CAKV,
                          dst_for=lambda m: wideK2[:, m, :] if m < KC else None))
        h1 = layernorm(xT, "ln1_g", "ln1_b", pairs=True, odt=f8)
        wideQ = act.tile([P, KC, R], f16, name="wideQ")
        wideK = act.tile([P, KC, R], f16, name="wideK")
        gemm8(Wqkv, h1, 3 * C,
              evict_store(0, "qkv_b", DSC_QKV,
                          dst_for=lambda m: wideQ[:, m, :] if m < KC
                          else (wideK[:, m - KC, :] if m < 2 * KC else None)),
              group_order=[2, 3, 4, 5, 0, 1])

        agbuf = act.tile([P, AGW], bf16, name="agbuf")
        o_sst, o_skf = 0, HD * NPAIR
        base2 = HD * NPAIR + NPAIR
        o_cst, o_ckf = base2, base2 + HD * NPAIR

        Kr_w = act.tile([P, KC, R], f16, name="Kr_w")
        K2r_w = act.tile([P, KC, R], f16, name="K2r_w")
        kfsW = tmp.tile([P, KC], f32, name="kfsW", bufs=1)
        kfsW2 = tmp.tile([P, KC], f32, name="kfsW2", bufs=1)
        Vn_l = [None] * NPAIR
        for h in (0, 1):
            ko = h * HKC
            elu1_w(wideK, ko)
            nc.vector.reduce_sum(kfsW[:, ko:ko + HKC],
                                 wideK[:, ko:ko + HKC, :], axis=AX.X)
            rope_w(wideK, ko, Kr_w[:, ko:ko + HKC, :])
            for p in range(ko, ko + HKC):
                Vn_l[p] = transpose_pair(
                    lambda n: go[2 * KC + p][:, n * P:n * P + RT[n]],
                    lambda n: f"Vn{p}_{n}", f16)
                Kn = transpose_pair(
                    lambda n: Kr_w[:, p, n * P:n * P + RT[n]],
                    lambda n: "t_kn", f16, bufs=2)
                kv_state(Kn, Vn_l[p],
                         agbuf[:, o_sst + p * HD:o_sst + (p + 1) * HD])
        nc.scalar.copy(agbuf[:, o_skf:o_skf + KC], kfsW[:])
        AGS = HD * NPAIR + NPAIR
        ag_in1 = dram.tile([P, AGS], bf16, name="ag_in1")
        ag_out1 = dram.tile([NC * P, AGS], bf16, name="ag_out1",
                            addr_space="Shared")
        nc.sync.dma_start(ag_in1[:], agbuf[:, 0:AGS])
        nc.gpsimd.collective_compute(
            "AllGather", OP.bypass,
            replica_groups=[list(range(NC))],
            ins=[ag_in1[:].opt()], outs=[ag_out1[:].opt()])
        for h in (0, 1):
            ko = h * HKC
            elu1_w(wideK2, ko)
            nc.vector.reduce_sum(kfsW2[:, ko:ko + HKC],
                                 wideK2[:, ko:ko + HKC, :], axis=AX.X)
            rope_w(wideK2, ko, K2r_w[:, ko:ko + HKC, :])
            for p in range(ko, ko + HKC):
                V2n = transpose_pair(
                    lambda n: go[4 * KC + p][:, n * P:n * P + RT[n]],
                    lambda n: "t_v2n", f16, bufs=2)
                K2n = transpose_pair(
                    lambda n: K2r_w[:, p, n * P:n * P + RT[n]],
                    lambda n: "t_kn", f16, bufs=2)
                kv_state(K2n, V2n,
                         agbuf[:, o_cst + p * HD:o_cst + (p + 1) * HD])
        nc.scalar.copy(agbuf[:, o_ckf:o_ckf + KC], kfsW2[:])

        # ---------- pre-AG: Q features + intra causal attention ----------
        Qr_w = act.tile([P, KC, R], f16, name="Qr_w")
        for h in (0, 1):
            elu1_w(wideQ, h * HKC)
            rope_w(wideQ, h * HKC, Qr_w[:, h * HKC:(h + 1) * HKC, :])
        Qf_w = wideQ
        yi_l = [None] * NPAIR
        for p in range(NPAIR):
            yp = gps.tile([P, SPW], f32, name="gps")
            ams = {}
            for h0 in (0, HD):
                for n in range(NT):
                    pa = sps.tile([P, SPW], f32, name="sps")
                    nc.tensor.matmul(
                        pa[:RT[n], :R],
                        lhsT=Kr_w[h0:h0 + HD, p, n * P:n * P + RT[n]],
                        rhs=Qr_w[h0:h0 + HD, p, :],
                        start=True, stop=True)
                    am = tmp.tile([P, R], f16, name="attM", bufs=4)
                    nc.vector.tensor_mul(am[:RT[n], :], pa[:RT[n], :R],
                                         maskT[n][:RT[n], :])
                    ams[(h0, n)] = am
            for h0 in (0, HD):
                for n in range(NT):
                    nc.tensor.matmul(
                        yp[h0:h0 + HD, :R],
                        lhsT=Vn_l[p][n][:RT[n], h0:h0 + HD],
                        rhs=ams[(h0, n)][:RT[n], :],
                        start=(n == 0), stop=(n == NT - 1))
            yi = act.tile([P, R], f32, name=f"yi{p}")
            nc.scalar.copy(yi[:], yp[:, :R])
            yi_l[p] = yi

        # ============ AllGather part 2 (CA region) + reduces ============
        ag_in2 = dram.tile([P, AGW - AGS], bf16, name="ag_in2")
        ag_out2 = dram.tile([NC * P, AGW - AGS], bf16, name="ag_out2",
                            addr_space="Shared")
        nc.sync.dma_start(ag_in2[:], agbuf[:, AGS:AGW])
        nc.gpsimd.collective_compute(
            "AllGather", OP.bypass,
            replica_groups=[list(range(NC))],
            ins=[ag_in2[:].opt()], outs=[ag_out2[:].opt()])
        # note: AG-2 is issued here, after the K2 section; AG-1 is already in
        # flight, so the SA reduce below overlaps AG-2's transport.
        OSP = HD * NPAIR
        accP = act.tile([P, AGW], f32, name="accP")
        accT = act.tile([P, AGW], f32, name="accT")
        agr1_l = []
        for r in range(NC):
            agr = tmp.tile([P, AGS], bf16, name=f"agr1_{r}", bufs=1)
            nc.sync.dma_start(agr[:], ag_out1[r * P:(r + 1) * P, :])
            agr1_l.append(agr)
        nc.vector.memset(accP[:, 0:OSP], 0.0)
        nc.vector.memset(accT[:, OSP:AGS], 0.0)
        for r in range(NC):
            nc.vector.scalar_tensor_tensor(accP[:, 0:OSP], agr1_l[r][:, 0:OSP],
                                           wsel[:, r:r + 1],
                                           accP[:, 0:OSP], op0=OP.mult, op1=OP.add)
        for r in range(NC):
            nc.vector.scalar_tensor_tensor(accT[:, OSP:AGS],
                                           agr1_l[r][:, OSP:AGS],
                                           wsel[:, NC + r:NC + r + 1],
                                           accT[:, OSP:AGS],
                                           op0=OP.mult, op1=OP.add)
        agr2_l = []
        for r in range(NC):
            agr = tmp.tile([P, AGW - AGS], bf16, name=f"agr2_{r}", bufs=1)
            nc.sync.dma_start(agr[:], ag_out2[r * P:(r + 1) * P, :])
            agr2_l.append(agr)
        nc.vector.memset(accT[:, AGS:AGW], 0.0)
        for r in range(NC):
            nc.vector.scalar_tensor_tensor(accT[:, AGS:AGW], agr2_l[r][:],
                                           wsel[:, NC + r:NC + r + 1],
                                           accT[:, AGS:AGW],
                                           op0=OP.mult, op1=OP.add)

        accPm = act.tile([P, AGW], f16, name="accPm")
        nc.scalar.copy(accPm[:, 0:OSP], accP[:, 0:OSP])
        accTm = act.tile([P, AGW], f16, name="accTm")
        nc.scalar.copy(accTm[:, OSP:AGS], accT[:, OSP:AGS])
        nc.scalar.copy(accTm[:, AGS:AGW], accT[:, AGS:AGW])

        # kf2: zero-padded per-head-half Kf-sum columns, [128, 2] per pair
        # (SA pairs at cols 2p, CA pairs at cols 2*NPAIR + 2p)
        kf2 = act.tile([P, 4 * NPAIR], f16, name="kf2")
        nc.vector.memset(kf2[:], 0.0)
        for p in range(NPAIR):
            c = o_skf + p
            nc.scalar.copy(kf2[0:HD, 2 * p:2 * p + 1], accTm[0:HD, c:c + 1])
            nc.scalar.copy(kf2[HD:P, 2 * p + 1:2 * p + 2], accTm[HD:P, c:c + 1])
        for p in range(NPAIR):
            c = o_ckf + p
            b = 2 * NPAIR
            nc.scalar.copy(kf2[0:HD, b + 2 * p:b + 2 * p + 1],
                           accTm[0:HD, c:c + 1])
            nc.scalar.copy(kf2[HD:P, b + 2 * p + 1:b + 2 * p + 2],
                           accTm[HD:P, c:c + 1])

        # ================= self attention =================
        def divide_and_pack(yp, Qf, kfbase, dst_ap, add=None):
            # den rows [2, R] = per-half Qf . kf_sum; reciprocal (x AY) then
            # broadcast to [128, R] via the Ea expander matmul
            dps = sps.tile([P, SPW], f32, name="sps")
            nc.tensor.matmul(dps[0:2, :R], lhsT=kf2[:, kfbase:kfbase + 2],
                             rhs=Qf[:], start=True, stop=True)
            rsf = tmp.tile([2, R], f32, name="d_rsf", bufs=2)
            nc.vector.reciprocal_approx_fast(rsf[:], dps[0:2, :R])
            rs16 = tmp.tile([2, R], f16, name="d_rs16", bufs=2)
            nc.scalar.mul(rs16[:], rsf[:], AY)
            denb = sps.tile([P, SPW], f32, name="sps")
            nc.tensor.matmul(denb[:, :R], lhsT=Ea[:], rhs=rs16[:],
                             start=True, stop=True)
            if add is not None:
                ys = tmp.tile([P, R], f32, name="ysum", bufs=2)
                nc.vector.tensor_add(ys[:], yp[:, :R], add[:])
                nc.vector.tensor_mul(dst_ap, ys[:], denb[:, :R])
            else:
                ys = tmp.tile([P, R], f32, name="ysum", bufs=2)
                nc.scalar.copy(ys[:], yp[:, :R])
                nc.vector.tensor_mul(dst_ap, ys[:], denb[:, :R])

        ySA = [act.tile([P, 2, R], f8, name=f"ySA{i}") for i in range(NPAIR // 2)]
        for p in range(NPAIR):
            c0 = o_sst + p * HD
            bd = tmp.tile([P, P], f16, name="bd", bufs=2)
            nc.vector.memset(bd[:], 0.0)
            nc.scalar.copy(bd[0:HD, 0:HD], accPm[0:HD, c0:c0 + HD])
            nc.scalar.copy(bd[HD:P, HD:P], accPm[HD:P, c0:c0 + HD])
            yp = gps.tile([P, SPW], f32, name="gps")
            nc.tensor.matmul(yp[:, :R], lhsT=bd[:], rhs=Qr_w[:, p, :],
                             start=True, stop=True)
            divide_and_pack(yp, Qf_w[:, p, :], 2 * p,
                            ySA[p // 2][:, p % 2, :], add=yi_l[p])

        x1T = [None] * KC

        def evict_res8(dst, bname, descale, res, rname):
            def ev(m, ps):
                d = act.tile([P, R], f32, name=rname(m), bufs=2)
                t = tmp.tile([P, R], f32, name="ev_t", bufs=2)
                nc.vector.tensor_scalar(t[:], ps, descale, pcol(bname, m),
                                        op0=OP.mult, op1=OP.add)
                nc.vector.tensor_add(d[:], t[:], res[m][:])
                dst[m] = d
            return ev

        gemm8(Wsap, ySA, C, evict_res8(x1T, "sap_b", DSC_SAP, xT,
                                       lambda k: f"res{k}"))
        dump("x1T", x1T)

        # ================= cross attention =================
        h2 = layernorm(x1T, "ln2_g", "ln2_b", pairs=True, odt=f8)
        wideQ2 = act.tile([P, KC, R], f16, name="wideQ2")
        gemm8(Wcaq, h2, C,
              evict_store(4 * KC, "caq_b", DSC_CAQ,
                          dst_for=lambda m: wideQ2[:, m, :]))
        Q2r_w = act.tile([P, KC, R], f16, name="Q2r_w")
        for h in (0, 1):
            elu1_w(wideQ2, h * HKC)
            rope_w(wideQ2, h * HKC, Q2r_w[:, h * HKC:(h + 1) * HKC, :])
        Q2f_w = wideQ2
        yCA = [act.tile([P, 2, R], f8, name=f"yCA{i}") for i in range(NPAIR // 2)]
        for p in range(NPAIR):
            c0 = o_cst + p * HD
            bd = tmp.tile([P, P], f16, name="bd", bufs=2)
            nc.vector.memset(bd[:], 0.0)
            nc.scalar.copy(bd[0:HD, 0:HD], accTm[0:HD, c0:c0 + HD])
            nc.scalar.copy(bd[HD:P, HD:P], accTm[HD:P, c0:c0 + HD])
            yp = gps.tile([P, SPW], f32, name="gps")
            nc.tensor.matmul(yp[:, :R], lhsT=bd[:], rhs=Q2r_w[:, p, :],
                             start=True, stop=True)
            divide_and_pack(yp, Q2f_w[:, p, :], 2 * NPAIR + 2 * p,
                            yCA[p // 2][:, p % 2, :])

        x2T = [None] * KC
        gemm8(Wcap, yCA, C, evict_res8(x2T, "cap_b", DSC_CAP, x1T,
                                       lambda k: f"res{k}"))
        dump("x2T", x2T)

        # ================= MLP (fp16) =================
        h3 = layernorm(x2T, "ln3_g", "ln3_b", pairs=False, odt=f16)
        gT = [None] * (4 * KC)

        def evict_gelu(m, ps):
            d = act.tile([P, R], f16, name=f"go{m}")
            nc.scalar.activation(d[:], ps, AF.Gelu_apprx_tanh,
                                 bias=pcol("fc_b", m))
            gT[m] = d
        gemm16(Wfc, h3, 4 * C, evict_gelu)

        xoT = [None] * KC

        def evict_res16(dst, bname, res, rname):
            def ev(m, ps):
                d = act.tile([P, R], f32, name=rname(m), bufs=2)
                nc.vector.scalar_tensor_tensor(d[:], ps, pcol(bname, m),
                                               res[m][:], op0=OP.add, op1=OP.add)
                dst[m] = d
            return ev

        gemm16(Wfcp, gT, C, evict_res16(xoT, "fcp_b", x2T, lambda k: f"res{k}"))

        # ================= transpose back + store =================
        for n in range(NT):
            onat = tmp.tile([P, C], f32, name="nat", bufs=2)
            for k in range(KC):
                pt = sps.tile([P, SPW], f32, name="sps")
                nc.tensor.transpose(pt[:RT[n], :P],
                                    xoT[k][:, n * P:n * P + RT[n]],
                                    ident[:, :])
                nc.scalar.copy(onat[:RT[n], k * P:(k + 1) * P],
                               pt[:RT[n], :P])
            nc.sync.dma_start(out_d[n * P:n * P + RT[n], :], onat[:RT[n], :])

    nc.compile()
    return nc


# ---------------------------------------------------------------------------
# Entry point
# ---------------------------------------------------------------------------

_CACHE = {}


def _get_program(cfg: Cfg):
    if cfg not in _CACHE:
        _CACHE[cfg] = build_program(cfg)
    return _CACHE[cfg]


def run(inputs, cfg: Cfg = Cfg(), trace: bool = False):
    from concourse.bass_utils import run_bass_kernel_spmd
    nc = _get_program(cfg)
    in_maps = _host_inputs(cfg, inputs)
    res = run_bass_kernel_spmd(nc, in_maps, core_ids=list(range(cfg.NCORE)),
                               trace=trace)
    outs = [res.results[c]["out"] for c in range(cfg.NCORE)]
    full = np.concatenate(outs, axis=0).reshape(cfg.B, cfg.T, cfg.C)
    return np.asarray(full, np.float32), res


def kernel(**inputs):
    out, _ = run(inputs)
    return out
